# revision 4
# baseline (speedup 1.0000x reference)
"""Trainium2 Bass kernel for single-head attention (no V projection).

Reference computation (per batch b):
    qk   = x @ W_qk.T + b_qk          # [n, 2d]
    q, k = qk[:, :d], qk[:, d:]
    dots[i, j] = k_i . q_j / sqrt(d)
    attn = softmax(dots, axis=-1)
    out[i] = sum_j attn[i, j] * x[j]

Sharding: data-parallel over batch b (8 batches -> 8 NeuronCores), no
collectives.

Algebraic rewrite (host side): dots = (x Wk^T + bk)(x Wq^T + bq)^T decomposes
into x (Wk^T Wq) x^T plus a row term (softmax-invariant, dropped), a column
term c_j = (Wq^T bk) . x_j (kept, folded into the exp bias), and a constant
(dropped).  So the device only computes, per batch:

  A:  P^T[d, i] = W2^T-chunks^T @ x^T      (W2 = Wk^T Wq * 1/sqrt(d), bf16)
  B:  S^T[j, i] = x^T-blocks^T @ P^T;  E^T = exp(S^T + c_j)  (ACT, bf16 out)
  C:  out[i, :] = E^T-blocks^T @ x accumulated over j in PSUM; softmax
      denominator from a DVE running sum of E^T strips + one tiny ones-matmul
      per 128-row block; normalize via per-partition reciprocal (DVE).

All three stages are fused per 512-column i-chunk so the PE never idles
(A'(c) -> B(c) -> C(c) back to back; DMA, ACT, and DVE run in the shadow).
All matmul operands are bf16 (full PE rate, half the SBUF/DMA of fp32), with
fp32 PSUM accumulation; exp is computed without max-subtraction (scores are
~N(0, 0.67) after scaling, so no overflow).

Host-side input layouts (computed in kernel()):
  xt [128, 8*2048]: xt[p, k*2048+i] = x[i, k*128+p]   (x^T, k-chunked)
  w2 [128, 8*8*128]: w2[p, ((m*8)+k)*128+c] = W2[k*128+p, m*128+c]
  xn [2048, 1024], cv [128, 16] (column bias, fp32), ones [128, 8].
"""
import sys

try:
    import concourse.bass as bass  # noqa: F401
except ImportError:  # pragma: no cover
    sys.path.insert(0, "/opt/trn_rl_repo")

import numpy as np
import ml_dtypes
import concourse.bass as bass
import concourse.mybir as mybir
import concourse.tile as tile
from concourse import bacc
from concourse.bass_utils import run_bass_kernel_spmd

B, N, D = 8, 2048, 1024
NCORES = 8
SCALE = 1.0 / np.sqrt(D)  # 1/32
BF16 = ml_dtypes.bfloat16

KD = D // 128   # 8 contraction chunks over d
MD = D // 128   # 8 output row-blocks of P^T
NJ = N // 128   # 16 key blocks (j)
CH = 512        # i-chunk width
NCH = N // CH   # 4 chunks
NSUB = CH // 128

_NC = None
LAST_RESULTS = None


def _build_nc():
    BF = mybir.dt.bfloat16
    F = mybir.dt.float32
    nc = bacc.Bacc("TRN2", target_bir_lowering=False, debug=False,
                   num_devices=NCORES)

    xt_d = nc.dram_tensor("xt", [128, KD * N], BF, kind="ExternalInput").ap()
    xn_d = nc.dram_tensor("xn", [N, D], BF, kind="ExternalInput").ap()
    w2_d = nc.dram_tensor("w2", [128, MD * KD * 128], BF,
                          kind="ExternalInput").ap()
    cv_d = nc.dram_tensor("cv", [128, NJ], F, kind="ExternalInput").ap()
    ones_d = nc.dram_tensor("ones", [128, 8], BF, kind="ExternalInput").ap()
    out_d = nc.dram_tensor("out", [N, D], F, kind="ExternalOutput").ap()

    with tile.TileContext(nc) as tc:
        with tc.tile_pool(name="xts", bufs=1) as xtp, \
             tc.tile_pool(name="xnp", bufs=1) as xnp, \
             tc.tile_pool(name="w2p", bufs=1) as w2p, \
             tc.tile_pool(name="ptp", bufs=1) as ptp, \
             tc.tile_pool(name="ep", bufs=1) as ep, \
             tc.tile_pool(name="esp", bufs=1) as esp, \
             tc.tile_pool(name="misc", bufs=1) as misc, \
             tc.tile_pool(name="ost", bufs=2) as ostp, \
             tc.tile_pool(name="psa", bufs=1, space="PSUM") as psa, \
             tc.tile_pool(name="psb", bufs=1, space="PSUM") as psb, \
             tc.tile_pool(name="psd", bufs=2, space="PSUM") as psd:

            xts = xtp.tile([128, KD * N], BF, tag="xts", name="xts")
            xtr = xts.rearrange("p (k i) -> p k i", k=KD)
            xtd_r = xt_d.rearrange("p (k i) -> p k i", k=KD)
            w2t = w2p.tile([128, MD * KD * 128], BF, tag="w2t", name="w2t")
            ptt = ptp.tile([128, KD * N], BF, tag="ptt", name="ptt")
            ptr = ptt.rearrange("p (k i) -> p k i", k=KD)

            # ---------------- DMA schedule ----------------
            # First-needed pieces first: W2 m=0 (split per k) and xT chunk 0.
            for k in range(KD):
                nc.sync.dma_start(out=w2t[:, k * 128:(k + 1) * 128],
                                  in_=w2_d[:, k * 128:(k + 1) * 128])
            for k in range(KD):
                nc.sync.dma_start(out=xtr[:, k, 0:CH], in_=xtd_r[:, k, 0:CH])
            cvt = misc.tile([128, NJ], F, tag="cv", name="cvt")
            nc.sync.dma_start(out=cvt, in_=cv_d)
            onest = misc.tile([128, 8], BF, tag="ones", name="onest")
            nc.sync.dma_start(out=onest, in_=ones_d)
            for m in range(1, MD):
                nc.sync.dma_start(
                    out=w2t[:, m * KD * 128:(m + 1) * KD * 128],
                    in_=w2_d[:, m * KD * 128:(m + 1) * KD * 128])
            for cc in range(1, NCH):
                nc.sync.dma_start(out=xtr[:, :, cc * CH:(cc + 1) * CH],
                                  in_=xtd_r[:, :, cc * CH:(cc + 1) * CH])
            xv = []
            for j in range(NJ):
                t = xnp.tile([128, D], BF, tag=f"xv{j}", name=f"xv{j}")
                nc.sync.dma_start(out=t, in_=xn_d[j * 128:(j + 1) * 128, :])
                xv.append(t)

            # ---------------- fused A' -> B -> C per i-chunk ----------------
            for c in range(NCH):
                ccols = slice(c * CH, (c + 1) * CH)

                # A': P^T[:, ccols]
                for m in range(MD):
                    pa = psa.tile([128, CH], F, tag=f"t{m % 4}",
                                  name=f"psa{m % 4}")
                    for k in range(KD):
                        w = (m * KD + k) * 128
                        nc.tensor.matmul(pa, w2t[:, w:w + 128],
                                         xtr[:, k, ccols],
                                         start=(k == 0), stop=(k == KD - 1))
                    nc.vector.tensor_copy(ptr[:, m, ccols], pa)

                # B: E^T strips for this chunk + running denominator sum
                es = []
                esum = esp.tile([128, CH], F, tag="esum", name="esum")
                for j in range(NJ):
                    pb = psb.tile([128, CH], F, tag=f"b{j % 2}",
                                  name=f"psb{j % 2}")
                    for k in range(KD):
                        nc.tensor.matmul(pb,
                                         xtr[:, k, j * 128:(j + 1) * 128],
                                         ptr[:, k, ccols],
                                         start=(k == 0), stop=(k == KD - 1))
                    ej = ep.tile([128, CH], BF, tag=f"e{j}", name=f"e{j}")
                    nc.scalar.activation(ej, pb,
                                         mybir.ActivationFunctionType.Exp,
                                         bias=cvt[:, j:j + 1], scale=1.0)
                    es.append(ej)
                    if j == 0:
                        nc.vector.tensor_copy(esum, ej)
                    else:
                        nc.vector.tensor_add(esum, esum, ej)
                esumR = esp.tile([128, CH], BF, tag="esumR", name="esumR")
                nc.vector.tensor_copy(esumR, esum)

                # C: out rows for this chunk
                for sub in range(NSUB):
                    icols = slice(sub * 128, (sub + 1) * 128)
                    p0 = psa.tile([128, CH], F, tag=f"t{(sub % 2) * 2}",
                                  name=f"psa{(sub % 2) * 2}")
                    p1 = psa.tile([128, CH], F, tag=f"t{(sub % 2) * 2 + 1}",
                                  name=f"psa{(sub % 2) * 2 + 1}")
                    for j in range(NJ):
                        lhs = es[j][:, icols]
                        nc.tensor.matmul(p0, lhs, xv[j][:, 0:CH],
                                         start=(j == 0), stop=(j == NJ - 1))
                        nc.tensor.matmul(p1, lhs, xv[j][:, CH:D],
                                         start=(j == 0), stop=(j == NJ - 1))
                    pd = psd.tile([128, 8], F, tag="pd", name="pd")
                    nc.tensor.matmul(pd, esumR[:, icols], onest,
                                     start=True, stop=True)
                    rden = ostp.tile([128, 1], F, tag="rden", name="rden")
                    nc.vector.reciprocal(rden, pd[:, 0:1])
                    ob = ostp.tile([128, D], F, tag="ob", name="ob")
                    nc.vector.tensor_scalar_mul(ob[:, 0:CH], p0, rden)
                    nc.vector.tensor_scalar_mul(ob[:, CH:D], p1, rden)
                    row = c * CH + sub * 128
                    nc.sync.dma_start(out=out_d[row:row + 128, :], in_=ob)

    nc.finalize()
    return nc


def _get_nc():
    global _NC
    if _NC is None:
        _NC = _build_nc()
    return _NC


def _prep_shared(W_qk, b_qk):
    W = np.ascontiguousarray(W_qk, dtype=np.float32)
    b = np.asarray(b_qk, dtype=np.float32).reshape(2 * D)
    Wq, Wk = W[:D], W[D:]
    W2s = (Wk.T @ Wq) * SCALE
    # w2[p, ((m*8)+k)*128+c] = W2s[k*128+p, m*128+c]
    w2l = np.ascontiguousarray(
        W2s.reshape(KD, 128, MD, 128).transpose(1, 2, 0, 3)
        .reshape(128, MD * KD * 128)).astype(BF16)
    wqb = (Wq.T @ b[D:]) * SCALE          # column-bias weight [D]
    ones = np.ones((128, 8), dtype=BF16)
    return w2l, wqb, ones


def _host_inputs(x_b, w2l, wqb, ones):
    xT = np.ascontiguousarray(x_b.T)      # [D, N] fp32
    xt = np.ascontiguousarray(
        xT.reshape(KD, 128, N).transpose(1, 0, 2).reshape(128, KD * N)
    ).astype(BF16)
    xn = np.ascontiguousarray(x_b).astype(BF16)
    cv = np.ascontiguousarray(
        (x_b @ wqb).astype(np.float32).reshape(NJ, 128).T)
    return {"xt": xt, "xn": xn, "w2": w2l, "cv": cv, "ones": ones}


def kernel(x: np.ndarray, W_qk: np.ndarray, b_qk: np.ndarray) -> np.ndarray:
    global LAST_RESULTS
    assert x.shape == (B, N, D), x.shape
    nc = _get_nc()

    x = np.ascontiguousarray(x, dtype=np.float32)
    w2l, wqb, ones = _prep_shared(W_qk, b_qk)
    in_maps = [_host_inputs(x[c], w2l, wqb, ones) for c in range(NCORES)]

    res = run_bass_kernel_spmd(nc, in_maps, core_ids=list(range(NCORES)))
    LAST_RESULTS = res
    out = np.stack([res.results[c]["out"] for c in range(NCORES)], axis=0)
    return out.astype(np.float32)


if __name__ == "__main__":
    rng = np.random.default_rng(0)
    x = rng.standard_normal((B, N, D), dtype=np.float32)
    limit = float(np.sqrt(6.0 / (D + 2 * D)))
    W = rng.uniform(-limit, limit, size=(2 * D, D)).astype(np.float32)
    b = np.zeros((2 * D,), dtype=np.float32)
    got = kernel(x, W, b)
    print("out", got.shape, got.dtype)


# revision 16
# speedup vs baseline: 1.0027x; 1.0027x over previous
"""Trainium2 Bass kernel for single-head attention (no V projection).

Reference computation (per batch b):
    qk   = x @ W_qk.T + b_qk          # [n, 2d]
    q, k = qk[:, :d], qk[:, d:]
    dots[i, j] = k_i . q_j / sqrt(d)
    attn = softmax(dots, axis=-1)
    out[i] = sum_j attn[i, j] * x[j]

Sharding: data-parallel over batch b (8 batches -> 8 NeuronCores), no
collectives.

Algebraic rewrite (host side): dots = (x Wk^T + bk)(x Wq^T + bq)^T decomposes
into x (Wk^T Wq) x^T plus a row term (softmax-invariant, dropped), a column
term c_j = (Wq^T bk) . x_j (kept, folded into the exp bias), and a constant
(dropped).  So the device only computes, per batch:

  A:  P^T[d, i] = W2^T-chunks^T @ x^T      (W2 = Wk^T Wq, bf16)
  B:  S^T[j, i] = x^T-blocks^T @ P^T       (bf16)
      E^T = exp(S^T/sqrt(d) + c_j)         (ACT, bf16 out)
  C:  out[i, :] = E^T-blocks^T @ x accumulated over j in PSUM; softmax
      denominator from a DVE running sum of E^T strips + one tiny ones-matmul
      per 128-row block; normalize via per-partition reciprocal (DVE).

All three stages are fused per 512-column i-chunk so the PE never idles
(A'(c) -> B(c) -> C(c) back to back; DMA, ACT, and DVE run in the shadow).
A PE warmup block (zero matmuls) runs during the DMA lead-in so the tensor
engine is at full clock when real work arrives.  All matmul operands are
bf16 (full PE rate, half the SBUF/DMA of fp32) with fp32 PSUM accumulation.
fp8 DoubleRow was measured 2x faster on stage B but fails the accuracy
budget (exp-amplified score tails / direct value quantization); bf16 is the
precision floor for this tolerance.  exp is computed without
max-subtraction (scaled scores are ~N(0, 0.9), no overflow).

Host-side input layouts (computed in kernel()):
  xt  [128, 8*2048] bf16: xt[p, k*2048+i] = x[i, k*128+p]   (x^T, k-chunked)
  w2  [128, 8*8*128] bf16: w2[p, ((m*8)+k)*128+c] = W2[k*128+p, m*128+c]
  xn  [2048, 1024] bf16, cv [128, 16] fp32 (column bias), ones [128, 8] bf16.
"""
import sys

try:
    import concourse.bass as bass  # noqa: F401
except ImportError:  # pragma: no cover
    sys.path.insert(0, "/opt/trn_rl_repo")

import numpy as np
import ml_dtypes
import concourse.bass as bass
import concourse.mybir as mybir
import concourse.tile as tile
from concourse import bacc
from concourse.bass_utils import run_bass_kernel_spmd

B, N, D = 8, 2048, 1024
NCORES = 8
SCALE = 1.0 / np.sqrt(D)  # 1/32
BF16 = ml_dtypes.bfloat16

KD = D // 128   # 8 contraction chunks over d
MD = D // 128   # 8 output row-blocks of P^T
NJ = N // 128   # 16 key blocks (j)
CH = 512        # i-chunk width
NCH = N // CH   # 4 chunks
NSUB = CH // 128
NWARM = 10      # PE warmup matmuls (fill the DMA lead-in, absorb clock ramp)

_NC = None
LAST_RESULTS = None


def _build_nc():
    BF = mybir.dt.bfloat16
    F = mybir.dt.float32
    nc = bacc.Bacc("TRN2", target_bir_lowering=False, debug=False,
                   num_devices=NCORES)

    xt_d = nc.dram_tensor("xt", [128, KD * N], BF, kind="ExternalInput").ap()
    xn_d = nc.dram_tensor("xn", [N, D], BF, kind="ExternalInput").ap()
    w2_d = nc.dram_tensor("w2", [128, MD * KD * 128], BF,
                          kind="ExternalInput").ap()
    cv_d = nc.dram_tensor("cv", [128, NJ], F, kind="ExternalInput").ap()
    ones_d = nc.dram_tensor("ones", [128, 8], BF, kind="ExternalInput").ap()
    out_d = nc.dram_tensor("out", [N, D], BF, kind="ExternalOutput").ap()

    with tile.TileContext(nc) as tc:
        with tc.tile_pool(name="xts", bufs=1) as xtp, \
             tc.tile_pool(name="xnp", bufs=1) as xnp, \
             tc.tile_pool(name="w2p", bufs=1) as w2p, \
             tc.tile_pool(name="ptp", bufs=1) as ptp, \
             tc.tile_pool(name="ep", bufs=1) as ep, \
             tc.tile_pool(name="esp", bufs=1) as esp, \
             tc.tile_pool(name="misc", bufs=1) as misc, \
             tc.tile_pool(name="ost", bufs=2) as ostp, \
             tc.tile_pool(name="psa", bufs=1, space="PSUM") as psa, \
             tc.tile_pool(name="psb", bufs=1, space="PSUM") as psb, \
             tc.tile_pool(name="psd", bufs=2, space="PSUM") as psd:

            xts = xtp.tile([128, KD * N], BF, tag="xts", name="xts")
            xtr = xts.rearrange("p (k i) -> p k i", k=KD)
            xtd_r = xt_d.rearrange("p (k i) -> p k i", k=KD)
            w2t = w2p.tile([128, MD * KD * 128], BF, tag="w2t", name="w2t")
            ptt = ptp.tile([128, KD * N], BF, tag="ptt", name="ptt")
            ptr = ptt.rearrange("p (k i) -> p k i", k=KD)

            # ---------------- PE warmup (no DMA dependencies) ----------------
            wml = misc.tile([128, 128], BF, tag="wml", name="wml")
            wmr = misc.tile([128, CH], BF, tag="wmr", name="wmr")
            nc.vector.memset(wml, 0.0)
            nc.vector.memset(wmr, 0.0)
            pw = psb.tile([128, CH], F, tag="b0", name="psb0")
            for w in range(NWARM):
                nc.tensor.matmul(pw, wml, wmr, start=(w == 0),
                                 stop=(w == NWARM - 1))

            # ---------------- DMA schedule ----------------
            # First-needed pieces first: W2 m=0 (split per k) and xT chunk 0.
            for k in range(KD):
                nc.sync.dma_start(out=w2t[:, k * 128:(k + 1) * 128],
                                  in_=w2_d[:, k * 128:(k + 1) * 128])
            for k in range(KD):
                nc.sync.dma_start(out=xtr[:, k, 0:CH], in_=xtd_r[:, k, 0:CH])
            cvt = misc.tile([128, NJ], F, tag="cv", name="cvt")
            nc.sync.dma_start(out=cvt, in_=cv_d)
            onest = misc.tile([128, 8], BF, tag="ones", name="onest")
            nc.sync.dma_start(out=onest, in_=ones_d)
            for m in range(1, MD):
                nc.sync.dma_start(
                    out=w2t[:, m * KD * 128:(m + 1) * KD * 128],
                    in_=w2_d[:, m * KD * 128:(m + 1) * KD * 128])
            for cc in range(1, NCH):
                nc.sync.dma_start(out=xtr[:, :, cc * CH:(cc + 1) * CH],
                                  in_=xtd_r[:, :, cc * CH:(cc + 1) * CH])
            xv = []
            for j in range(NJ):
                t = xnp.tile([128, D], BF, tag=f"xv{j}", name=f"xv{j}")
                nc.sync.dma_start(out=t, in_=xn_d[j * 128:(j + 1) * 128, :])
                xv.append(t)

            # ---------------- fused A' -> B -> C per i-chunk ----------------
            for c in range(NCH):
                ccols = slice(c * CH, (c + 1) * CH)

                # A': P^T[:, ccols]  (bf16 x bf16 -> fp8 store for stage B)
                for m in range(MD):
                    pa = psa.tile([128, CH], F, tag=f"t{m % 4}",
                                  name=f"psa{m % 4}")
                    for k in range(KD):
                        w = (m * KD + k) * 128
                        nc.tensor.matmul(pa, w2t[:, w:w + 128],
                                         xtr[:, k, ccols],
                                         start=(k == 0), stop=(k == KD - 1))
                    nc.vector.tensor_copy(ptr[:, m, ccols], pa)

                # B: E^T strips for this chunk + denom sum
                es = []
                esum = esp.tile([128, CH], F, tag="esum", name="esum")
                for j in range(NJ):
                    pb = psb.tile([128, CH], F, tag=f"b{j % 2}",
                                  name=f"psb{j % 2}")
                    for k in range(KD):
                        nc.tensor.matmul(pb,
                                         xtr[:, k, j * 128:(j + 1) * 128],
                                         ptr[:, k, ccols],
                                         start=(k == 0), stop=(k == KD - 1))
                    ej = ep.tile([128, CH], BF, tag=f"e{j}", name=f"e{j}")
                    nc.scalar.activation(ej, pb,
                                         mybir.ActivationFunctionType.Exp,
                                         bias=cvt[:, j:j + 1], scale=SCALE)
                    es.append(ej)
                    if j == 0:
                        nc.vector.tensor_copy(esum, ej)
                    else:
                        nc.vector.tensor_add(esum, esum, ej)
                esumR = esp.tile([128, CH], BF, tag="esumR", name="esumR")
                nc.vector.tensor_copy(esumR, esum)

                # C: out rows for this chunk (bf16)
                for sub in range(NSUB):
                    icols = slice(sub * 128, (sub + 1) * 128)
                    p0 = psa.tile([128, CH], F, tag=f"t{(sub % 2) * 2}",
                                  name=f"psa{(sub % 2) * 2}")
                    p1 = psa.tile([128, CH], F, tag=f"t{(sub % 2) * 2 + 1}",
                                  name=f"psa{(sub % 2) * 2 + 1}")
                    for j in range(NJ):
                        lhs = es[j][:, icols]
                        nc.tensor.matmul(p0, lhs, xv[j][:, 0:CH],
                                         start=(j == 0), stop=(j == NJ - 1))
                        nc.tensor.matmul(p1, lhs, xv[j][:, CH:D],
                                         start=(j == 0), stop=(j == NJ - 1))
                    pd = psd.tile([128, 8], F, tag="pd", name="pd")
                    nc.tensor.matmul(pd, esumR[:, icols], onest,
                                     start=True, stop=True)
                    rden = ostp.tile([128, 1], F, tag="rden", name="rden")
                    nc.vector.reciprocal(rden, pd[:, 0:1])
                    ob = ostp.tile([128, D], BF, tag="ob", name="ob")
                    row = c * CH + sub * 128
                    nc.vector.tensor_scalar_mul(ob[:, 0:CH], p0, rden)
                    nc.sync.dma_start(out=out_d[row:row + 128, 0:CH],
                                      in_=ob[:, 0:CH])
                    nc.vector.tensor_scalar_mul(ob[:, CH:D], p1, rden)
                    nc.sync.dma_start(out=out_d[row:row + 128, CH:D],
                                      in_=ob[:, CH:D])

    nc.finalize()
    return nc


def _get_nc():
    global _NC
    if _NC is None:
        _NC = _build_nc()
    return _NC


def _prep_shared(W_qk, b_qk):
    W = np.ascontiguousarray(W_qk, dtype=np.float32)
    b = np.asarray(b_qk, dtype=np.float32).reshape(2 * D)
    Wq, Wk = W[:D], W[D:]
    W2 = Wk.T @ Wq                        # raw scale (P, x are O(1) for fp8)
    # w2[p, ((m*8)+k)*128+c] = W2[k*128+p, m*128+c]
    w2l = np.ascontiguousarray(
        W2.reshape(KD, 128, MD, 128).transpose(1, 2, 0, 3)
        .reshape(128, MD * KD * 128)).astype(BF16)
    wqb = Wq.T @ b[D:]                    # column-bias weight [D] (raw)
    ones = np.ones((128, 8), dtype=BF16)
    return w2l, wqb, ones


def _host_inputs(x_b, w2l, wqb, ones):
    xT = np.ascontiguousarray(x_b.T)      # [D, N] fp32
    xtf = np.ascontiguousarray(
        xT.reshape(KD, 128, N).transpose(1, 0, 2).reshape(128, KD * N))
    xn = np.ascontiguousarray(x_b).astype(BF16)
    cv = np.ascontiguousarray(
        ((x_b @ wqb) * SCALE).astype(np.float32).reshape(NJ, 128).T)
    return {"xt": xtf.astype(BF16), "xn": xn,
            "w2": w2l, "cv": cv, "ones": ones}


def kernel(x: np.ndarray, W_qk: np.ndarray, b_qk: np.ndarray) -> np.ndarray:
    global LAST_RESULTS
    assert x.shape == (B, N, D), x.shape
    nc = _get_nc()

    x = np.ascontiguousarray(x, dtype=np.float32)
    w2l, wqb, ones = _prep_shared(W_qk, b_qk)
    in_maps = [_host_inputs(x[c], w2l, wqb, ones) for c in range(NCORES)]

    res = run_bass_kernel_spmd(nc, in_maps, core_ids=list(range(NCORES)))
    LAST_RESULTS = res
    out = np.stack([res.results[c]["out"] for c in range(NCORES)], axis=0)
    return out.astype(np.float32)


if __name__ == "__main__":
    rng = np.random.default_rng(0)
    x = rng.standard_normal((B, N, D), dtype=np.float32)
    limit = float(np.sqrt(6.0 / (D + 2 * D)))
    W = rng.uniform(-limit, limit, size=(2 * D, D)).astype(np.float32)
    b = np.zeros((2 * D,), dtype=np.float32)
    got = kernel(x, W, b)
    print("out", got.shape, got.dtype)
